# revision 1
# baseline (speedup 1.0000x reference)
"""Trainium2 Bass kernel for DariushFlashAttention2 (per-token [H,H] head
attention).

Math (per token position): reshape row of q/k/v [E=4096] -> [H=32, D=128];
  L = Q @ K^T / sqrt(D)           (32x32)
  W = softmax(L, axis=-1)
  O = W @ V                       (32x128)
Tokens (B*S = 8192) are independent -> shard 1024 tokens per NeuronCore
across 8 cores (data parallel, no collectives).

Per-core layout: tokens processed in 16 chunks of 64 tokens. SBUF chunk
tiles are [128, 2048] with partition p = 32*(token%4) + head and free
f = 128*quad + d, so every 4 consecutive tokens ("quad") stack their
[32,128] head matrices on the 4 partition groups. SWDGE (gpsimd) DMA
casts fp32 -> bf16 inline on load (the PE runs bf16 at 1 cycle/column vs
4 for fp32). A DVE 32x32 block-transpose yields per-(token, d-chunk)
[d, head] blocks in place, which feed PE 32x32 tile_position matmuls:
  mm1: 4 accumulating K=32 matmuls per token (d-chunks) on PE subarray
       (t, t) -> PSUM logits L[32t+h, g] (fp32).
  exp: one ScalarE activation per quad: E = exp(L/sqrt(D)) (bf16) with
       accum_out giving the softmax denominator Z per (t, h) for free.
  DVE block-transpose of E (4 quads batched) -> eT[g, h] blocks.
  mm2: per token one K=32, N=128 matmul (subarray (t, t)):
       O_unnorm[32t+h, d] = sum_g eT[g, h] * V[32t+g, d]  -> PSUM.
  epilogue: reciprocal(Z) then per-partition broadcast multiply
       (alternating DVE/ScalarE) into the fp32 output staging tile.

Measured (8-core HW, in-NEFF repeat-loop slope timing): 277 us per full
pass; cost model predicts 202 us with DMA 92% / ACT 70% / DVE 59% /
PE 55% busy. Ablations: removing 3/4 of the mm1 PE matmuls saves only
27 us, fully-contiguous DMA layouts save only 13 us -- the kernel is
evenly engine-constrained (ridge regime), so further gains need a
combined restructure (e.g. bf16 xbar DMA-transpose to enable full-K
mm1 with 4x fewer PE instructions AND lighter DVE), not local tweaks.
"""

import math

import numpy as np

NUM_CORES = 8
B, S, E = 2, 4096, 4096
H, D = 32, 128
T_TOTAL = B * S  # 8192 tokens
T_CORE = T_TOTAL // NUM_CORES  # 1024 tokens per core
CHUNK_TOKENS = 64  # tokens per chunk
N_CHUNKS = T_CORE // CHUNK_TOKENS  # 16
QUADS_PER_CHUNK = CHUNK_TOKENS // 4  # 16
INV_SQRT_D = 1.0 / math.sqrt(D)

_CACHE = {}


def _build_bass(n_chunks=N_CHUNKS, loop_reps=None, variant="base"):
    """variant: 'base', or timing-diagnostic variants (wrong numerics):
    'small_mm'  - mm1 uses 1 d-chunk instead of 4 (PE instr count 4x lower)
    'contig_dma'- q/k/v/out DMAs use flat contiguous layouts (max DMA eff)
    """
    import concourse.bacc as bacc
    import concourse.tile as tile
    from concourse import mybir

    fp32 = mybir.dt.float32
    bf16 = mybir.dt.bfloat16
    Exp = mybir.ActivationFunctionType.Exp
    Copy = mybir.ActivationFunctionType.Copy

    nc = bacc.Bacc()

    t_core = n_chunks * CHUNK_TOKENS
    q = nc.dram_tensor("q", [t_core, E], fp32, kind="ExternalInput")
    k = nc.dram_tensor("k", [t_core, E], fp32, kind="ExternalInput")
    v = nc.dram_tensor("v", [t_core, E], fp32, kind="ExternalInput")
    out = nc.dram_tensor("out", [t_core, E], fp32, kind="ExternalOutput")

    # [chunk, (t h), quad, d] views matching SBUF tiles with partition =
    # 32*t + head and free = 128*quad + d. The (t h) partition dim merges
    # into a single stride-128 dim, keeping DMA access patterns 3D.
    def chunk_view(x):
        if variant == "contig_dma":
            return x.rearrange("(c r) (a f) -> c (r a) f", c=n_chunks, r=64, a=2)
        return x.rearrange(
            "(c q t) (h d) -> c (t h) q d", c=n_chunks, q=QUADS_PER_CHUNK, t=4, h=32
        )

    qv, kv, vv, ov = map(chunk_view, (q, k, v, out))
    mm1_chunks = 1 if variant == "small_mm" else 4

    with tile.TileContext(nc) as tc:
        with (
            tc.tile_pool(name="big", bufs=2) as big,
            tc.tile_pool(name="small", bufs=4) as small,
            tc.tile_pool(name="psum_l", bufs=3, space="PSUM") as psum_l,
            tc.tile_pool(name="psum_o", bufs=4, space="PSUM") as psum_o,
            tc.For_i(0, loop_reps, 1) if loop_reps else _null_ctx(),
        ):
            for ch in range(n_chunks):
                qb = big.tile([128, 2048], bf16, tag="qb")
                kb = big.tile([128, 2048], bf16, tag="kb")
                vb = big.tile([128, 2048], bf16, tag="vb")
                # SWDGE DMA casts fp32 -> bf16 inline
                nc.gpsimd.dma_start(out=qb, in_=qv[ch])
                nc.gpsimd.dma_start(out=kb, in_=kv[ch])
                nc.gpsimd.dma_start(out=vb, in_=vv[ch])

                # DVE 32x32 block-transpose (same dtype)
                qt = big.tile([128, 2048], bf16, tag="qt")
                kt = big.tile([128, 2048], bf16, tag="kt")
                nc.vector.transpose(qt, qb)
                nc.vector.transpose(kt, kb)

                outc = big.tile([128, 2048], fp32, tag="outc")

                for g4 in range(QUADS_PER_CHUNK // 4):
                    e4 = small.tile([128, 128], bf16, tag="e4")
                    z4 = small.tile([128, 4], fp32, tag="z4")
                    # one PSUM bank per quad for mm2 output (each 32-row
                    # bank row gets exactly one start/stop matmul)
                    pos = [
                        psum_o.tile([128, 512], fp32, tag="po", name="po")
                        for _ in range(4)
                    ]

                    for j in range(4):
                        qq = 4 * g4 + j
                        # full-bank tile; mm1 uses cols 0:32 only
                        pl = psum_l.tile([128, 512], fp32, tag="pl")
                        for t in range(4):
                            p0 = 32 * t
                            for c in range(mm1_chunks):
                                f0 = 128 * qq + 32 * c
                                nc.tensor.matmul(
                                    pl[p0 : p0 + 32, 0:32],
                                    lhsT=qt[p0 : p0 + 32, f0 : f0 + 32],
                                    rhs=kt[p0 : p0 + 32, f0 : f0 + 32],
                                    start=(c == 0),
                                    stop=(c == mm1_chunks - 1),
                                    tile_position=(p0, p0),
                                )
                        # E = exp(L / sqrt(D)); Z[t,h] = sum_g E  (free axis)
                        nc.scalar.activation(
                            out=e4[:, 32 * j : 32 * j + 32],
                            in_=pl[:, 0:32],
                            func=Exp,
                            scale=INV_SQRT_D,
                            accum_out=z4[:, j : j + 1],
                        )

                    et4 = small.tile([128, 128], bf16, tag="et4")
                    nc.vector.transpose(et4, e4)

                    for j in range(4):
                        qq = 4 * g4 + j
                        for t in range(4):
                            p0 = 32 * t
                            nc.tensor.matmul(
                                pos[j][p0 : p0 + 32, 0:128],
                                lhsT=et4[p0 : p0 + 32, 32 * j : 32 * j + 32],
                                rhs=vb[p0 : p0 + 32, 128 * qq : 128 * qq + 128],
                                start=True,
                                stop=True,
                                tile_position=(p0, p0),
                            )

                    rz4 = small.tile([128, 4], fp32, tag="rz4")
                    nc.vector.reciprocal(rz4, z4)
                    for j in range(4):
                        qq = 4 * g4 + j
                        src = pos[j][:, 0:128]
                        dst = outc[:, 128 * qq : 128 * qq + 128]
                        sc = rz4[:, j : j + 1]
                        if j % 2 == 0:
                            nc.vector.tensor_scalar_mul(dst, src, sc)
                        else:
                            nc.scalar.activation(
                                out=dst, in_=src, func=Copy, scale=sc
                            )

                nc.sync.dma_start(out=ov[ch], in_=outc)

    nc.finalize()
    return nc


def _null_ctx():
    import contextlib

    return contextlib.nullcontext()


def get_nc(n_chunks=N_CHUNKS, loop_reps=None, variant="base"):
    key = ("nc", n_chunks, loop_reps, variant)
    if key not in _CACHE:
        _CACHE[key] = _build_bass(n_chunks, loop_reps, variant)
    return _CACHE[key]


def _build_warmup():
    """Tiny 8-core memcpy kernel used to shake out cold-device state before
    the first real execution (a fresh device has been observed to fail its
    very first heavy 8-core NEFF with EXEC_UNIT_UNRECOVERABLE)."""
    import concourse.bacc as bacc
    import concourse.tile as tile
    from concourse import mybir

    nc = bacc.Bacc()
    x = nc.dram_tensor("x", [128, 512], mybir.dt.float32, kind="ExternalInput")
    y = nc.dram_tensor("y", [128, 512], mybir.dt.float32, kind="ExternalOutput")
    with tile.TileContext(nc) as tc:
        with tc.tile_pool(name="p", bufs=1) as p:
            t = p.tile([128, 512], mybir.dt.float32, name="t")
            nc.sync.dma_start(out=t, in_=x[:, :])
            nc.sync.dma_start(out=y[:, :], in_=t)
    nc.finalize()
    return nc


def _warmup():
    from concourse.bass_utils import run_bass_kernel_spmd

    if "warm" in _CACHE:
        return
    nc = _build_warmup()
    x = np.zeros((128, 512), np.float32)
    try:
        run_bass_kernel_spmd(
            nc, [{"x": x} for _ in range(NUM_CORES)], core_ids=list(range(NUM_CORES))
        )
    except Exception:
        pass  # warmup is best-effort
    _CACHE["warm"] = True


def kernel(q, k, v, _trace=False):
    from concourse.bass_utils import run_bass_kernel_spmd

    q = np.ascontiguousarray(np.asarray(q, dtype=np.float32)).reshape(T_TOTAL, E)
    k = np.ascontiguousarray(np.asarray(k, dtype=np.float32)).reshape(T_TOTAL, E)
    v = np.ascontiguousarray(np.asarray(v, dtype=np.float32)).reshape(T_TOTAL, E)

    nc = get_nc()
    in_maps = []
    for c in range(NUM_CORES):
        sl = slice(c * T_CORE, (c + 1) * T_CORE)
        in_maps.append({"q": q[sl], "k": k[sl], "v": v[sl]})

    _warmup()
    res = None
    for attempt in range(3):
        try:
            res = run_bass_kernel_spmd(
                nc, in_maps, core_ids=list(range(NUM_CORES)), trace=_trace
            )
            break
        except Exception:
            if attempt == 2:
                raise
    outs = [r["out"] for r in res.results]
    full = np.concatenate(outs, axis=0).reshape(B, S, E).astype(np.float32)
    if _trace:
        return full, res
    return full



# revision 2
# speedup vs baseline: 1.6168x; 1.6168x over previous
"""Trainium2 Bass kernel for DariushFlashAttention2 (per-token [H,H] head
attention).

Math (per token position): reshape row of q/k/v [E=4096] -> [H=32, D=128];
  L = Q @ K^T / sqrt(D)           (32x32)
  W = softmax(L, axis=-1)
  O = W @ V                       (32x128)
Tokens (B*S = 8192) are independent -> shard 1024 tokens per NeuronCore
across 8 cores (data parallel, no collectives).

v2 design (instruction-count-minimal, DMA-roofline oriented):
- Host pre-casts q/k/v to bf16 (device math was already bf16), halving
  input HBM traffic; output is stored bf16 and upcast on host.
- Q and K load via HWDGE xbar DMA-transpose straight into d-major layout
  qT/kT [d=128, (token,h)] - no on-chip transposes at all.
- Per quad of 4 tokens, ONE K=128 matmul lhsT=kT rhs=qT computes the full
  cross-token [128,128] logit block L^T (off-diagonal = garbage).
- ONE ScalarE exp over 4 quads' logits [128,512], then ONE DVE multiply
  by a constant block-diagonal 0/1 mask kills the garbage -> block-diag
  E^T, bf16.
- V loads in natural [(t,g), d] layout interleaved with a ones column
  (stride 129); ONE matmul per token-quad lhsT=E^T rhs=[V|1] yields
  O_unnorm and the softmax denominator Z in PSUM cols [0:128]|[128].
- Epilogue: DVE reciprocal of Z, per-partition broadcast multiply
  (alternating DVE/ScalarE) into a bf16 staging tile, single DMA out.

Per core: 4 chunks x 256 tokens; ~1200 instructions total (PE 512,
ACT ~200, DVE ~450, DMA 17) vs ~8000 in v1 - per-instruction dispatch
overhead (~50-90ns) dominated v1's 5120 tiny PE matmuls.
"""

import math

import numpy as np

NUM_CORES = 8
B, S, E = 2, 4096, 4096
H, D = 32, 128
T_TOTAL = B * S  # 8192 tokens
T_CORE = T_TOTAL // NUM_CORES  # 1024 tokens per core
CHUNK_TOKENS = 256  # tokens per chunk
N_CHUNKS = T_CORE // CHUNK_TOKENS  # 4
QPC = CHUNK_TOKENS // 4  # quads per chunk = 64
INV_SQRT_D = 1.0 / math.sqrt(D)

_CACHE = {}


def _mask_np():
    import ml_dtypes

    m = np.zeros((128, 512), np.float32)
    for rep in range(4):
        for t in range(4):
            m[32 * t : 32 * t + 32, 128 * rep + 32 * t : 128 * rep + 32 * t + 32] = 1.0
    return m.astype(ml_dtypes.bfloat16)


def _build_bass(n_chunks=N_CHUNKS, loop_reps=None):
    import concourse.bacc as bacc
    import concourse.tile as tile
    from concourse import mybir

    fp32 = mybir.dt.float32
    bf16 = mybir.dt.bfloat16
    Exp = mybir.ActivationFunctionType.Exp
    Copy = mybir.ActivationFunctionType.Copy
    Mult = mybir.AluOpType.mult

    nc = bacc.Bacc()

    t_core = n_chunks * CHUNK_TOKENS
    q = nc.dram_tensor("q", [t_core, E], bf16, kind="ExternalInput")
    k = nc.dram_tensor("k", [t_core, E], bf16, kind="ExternalInput")
    v = nc.dram_tensor("v", [t_core, E], bf16, kind="ExternalInput")
    mk = nc.dram_tensor("mk", [128, 512], bf16, kind="ExternalInput")
    out = nc.dram_tensor("out", [t_core, E], bf16, kind="ExternalOutput")

    # 2D per-chunk views [(token h), d] for the xbar DMA-transpose loads.
    qv = q.rearrange("(c r) (h d) -> c (r h) d", c=n_chunks, h=H)
    kv = k.rearrange("(c r) (h d) -> c (r h) d", c=n_chunks, h=H)
    # V natural per-chunk view [(t g), quad, d]; token = c*256 + 4*quad + t.
    vv = v.rearrange("(c q t) (g d) -> c (t g) q d", c=n_chunks, t=4, g=H)
    ov = out.rearrange("(c q t) (h d) -> c (t h) q d", c=n_chunks, t=4, h=H)

    with tile.TileContext(nc) as tc:
        with (
            tc.tile_pool(name="cst", bufs=1) as cst,
            tc.tile_pool(name="big", bufs=2) as big,
            tc.tile_pool(name="small", bufs=4) as small,
            tc.tile_pool(name="psum_l", bufs=2, space="PSUM") as psum_l,
            tc.tile_pool(name="psum_o", bufs=4, space="PSUM") as psum_o,
        ):
            mkt = cst.tile([128, 512], bf16, name="mkt")
            nc.sync.dma_start(out=mkt, in_=mk[:, :])
            with tc.For_i(0, loop_reps, 1) if loop_reps else _null_ctx():
                for ch in range(n_chunks):
                    qT = big.tile([128, CHUNK_TOKENS * H], bf16, tag="qT")
                    kT = big.tile([128, CHUNK_TOKENS * H], bf16, tag="kT")
                    vx = big.tile([128, QPC, 129], bf16, tag="vx")
                    nc.sync.dma_start(out=qT, in_=qv[ch], transpose=True)
                    nc.sync.dma_start(out=kT, in_=kv[ch], transpose=True)
                    nc.sync.dma_start(out=vx[:, :, 0:128], in_=vv[ch])
                    nc.vector.memset(vx[:, :, 128:129], 1.0)

                    outw = big.tile([128, QPC * 128], bf16, tag="outw")

                    for j4 in range(QPC // 4):
                        pl = psum_l.tile([128, 512], fp32, tag="pl")
                        for j in range(4):
                            qq = 4 * j4 + j
                            f0 = 128 * qq
                            nc.tensor.matmul(
                                pl[:, 128 * j : 128 * j + 128],
                                lhsT=kT[:, f0 : f0 + 128],
                                rhs=qT[:, f0 : f0 + 128],
                                start=True,
                                stop=True,
                            )
                        eg = small.tile([128, 512], bf16, tag="eg")
                        nc.scalar.activation(
                            out=eg, in_=pl, func=Exp, scale=INV_SQRT_D
                        )
                        em = small.tile([128, 512], bf16, tag="em")
                        nc.vector.tensor_tensor(em, eg, mkt, op=Mult)

                        po = [
                            psum_o.tile([128, 258], fp32, tag="po", name="po")
                            for _ in range(2)
                        ]
                        for j in range(4):
                            qq = 4 * j4 + j
                            nc.tensor.matmul(
                                po[j // 2][:, 129 * (j % 2) : 129 * (j % 2) + 129],
                                lhsT=em[:, 128 * j : 128 * j + 128],
                                rhs=vx[:, qq, :],
                                start=True,
                                stop=True,
                            )
                        rz = small.tile([128, 4], fp32, tag="rz")
                        for j in range(4):
                            nc.vector.reciprocal(
                                rz[:, j : j + 1],
                                po[j // 2][:, 129 * (j % 2) + 128 : 129 * (j % 2) + 129],
                            )
                        for j in range(4):
                            qq = 4 * j4 + j
                            src = po[j // 2][:, 129 * (j % 2) : 129 * (j % 2) + 128]
                            dst = outw[:, 128 * qq : 128 * qq + 128]
                            sc = rz[:, j : j + 1]
                            if j % 2 == 0:
                                nc.vector.tensor_scalar_mul(dst, src, sc)
                            else:
                                nc.scalar.activation(
                                    out=dst, in_=src, func=Copy, scale=sc
                                )

                    nc.sync.dma_start(out=ov[ch], in_=outw)

    nc.finalize()
    return nc


def _null_ctx():
    import contextlib

    return contextlib.nullcontext()


def get_nc(n_chunks=N_CHUNKS, loop_reps=None):
    key = ("nc", n_chunks, loop_reps)
    if key not in _CACHE:
        _CACHE[key] = _build_bass(n_chunks, loop_reps)
    return _CACHE[key]


def _build_warmup():
    """Tiny 8-core memcpy kernel used to shake out cold-device state before
    the first real execution (a fresh device has been observed to fail its
    very first heavy 8-core NEFF with EXEC_UNIT_UNRECOVERABLE)."""
    import concourse.bacc as bacc
    import concourse.tile as tile
    from concourse import mybir

    nc = bacc.Bacc()
    x = nc.dram_tensor("x", [128, 512], mybir.dt.float32, kind="ExternalInput")
    y = nc.dram_tensor("y", [128, 512], mybir.dt.float32, kind="ExternalOutput")
    with tile.TileContext(nc) as tc:
        with tc.tile_pool(name="p", bufs=1) as p:
            t = p.tile([128, 512], mybir.dt.float32, name="t")
            nc.sync.dma_start(out=t, in_=x[:, :])
            nc.sync.dma_start(out=y[:, :], in_=t)
    nc.finalize()
    return nc


def _warmup():
    from concourse.bass_utils import run_bass_kernel_spmd

    if "warm" in _CACHE:
        return
    nc = _build_warmup()
    x = np.zeros((128, 512), np.float32)
    try:
        run_bass_kernel_spmd(
            nc, [{"x": x} for _ in range(NUM_CORES)], core_ids=list(range(NUM_CORES))
        )
    except Exception:
        pass  # warmup is best-effort
    _CACHE["warm"] = True


def kernel(q, k, v, _trace=False):
    import ml_dtypes
    from concourse.bass_utils import run_bass_kernel_spmd

    bf = ml_dtypes.bfloat16
    q = np.asarray(q, dtype=np.float32).reshape(T_TOTAL, E).astype(bf)
    k = np.asarray(k, dtype=np.float32).reshape(T_TOTAL, E).astype(bf)
    v = np.asarray(v, dtype=np.float32).reshape(T_TOTAL, E).astype(bf)
    mk = _mask_np()

    nc = get_nc()
    in_maps = []
    for c in range(NUM_CORES):
        sl = slice(c * T_CORE, (c + 1) * T_CORE)
        in_maps.append(
            {
                "q": np.ascontiguousarray(q[sl]),
                "k": np.ascontiguousarray(k[sl]),
                "v": np.ascontiguousarray(v[sl]),
                "mk": mk,
            }
        )

    _warmup()
    res = None
    for attempt in range(3):
        try:
            res = run_bass_kernel_spmd(
                nc, in_maps, core_ids=list(range(NUM_CORES)), trace=_trace
            )
            break
        except Exception:
            if attempt == 2:
                raise
    outs = [np.asarray(r["out"], dtype=np.float32) for r in res.results]
    full = np.concatenate(outs, axis=0).reshape(B, S, E)
    if _trace:
        return full, res
    return full
